# revision 6
# baseline (speedup 1.0000x reference)
"""Llama4-style MoE (top-1 routing, E=8) + shared SwiGLU expert on 8 Trainium2 cores.

Strategy (expert-parallel + data-parallel shared, host dispatch/combine):
  - Host computes router logits / top-1 routing / sigmoid scaling (0.03% of FLOPs)
    and sorts tokens by expert — the "dispatch" step of the sharding.
  - Core e gets: the tokens routed to expert e (zero-padded to a uniform CAP),
    expert e's SwiGLU weights, plus a 1/8 slice of all tokens and the replicated
    shared-expert weights.  Each core runs two SwiGLU MLPs (routed segment +
    shared segment) and the router-logits matmul for its shared slice.
  - All matmuls in float32r (fp32 storage, ~bf16 rate on the PE at free>=256).
  - Host packs every device input into the exact SBUF tile layout so each DMA
    is one contiguous >=8KB-per-partition read (DMA packet efficiency).
  - Host "combine": routed and shared partial outputs are summed and scattered
    back to the original token order.

Device layouts (P=128 partitions):
  xt   [P, 16, M]          x^T tokens:      xt[p, hc, m]  = x[m, hc*128+p]
  w1   [16, P, 16, P]      gate/up weights: w1[ig, p, hc, i] = W[ig*128+i, hc*128+p]
  w2   [16, P, 16, P]      down weights:    w2[hp, pi, ic, hj] = Wd[hp*128+hj, ic*128+pi]
  out  [H, M]              output^T (host transposes back)
"""

import numpy as np
from contextlib import ExitStack

import concourse.bacc as bacc
import concourse.tile as tile
from concourse import mybir
from concourse.bass_utils import run_bass_kernel_spmd

P = 128
H = 2048
I = 2048
E = 8
NCORES = 8
HC = H // P    # 16 contraction chunks (stage 1)
IC = I // P    # 16 i chunks

f32 = mybir.dt.float32
f32r = mybir.dt.float32r

_prog_cache: dict = {}


def _m_tiles(M, width=512):
    # float32r matmuls drop to 1/4 rate below a 256-wide moving operand, so
    # keep every tile >= 256 by rebalancing the tail against the previous tile.
    widths = []
    rem = M
    while rem > 0:
        w = min(width, rem)
        widths.append(w)
        rem -= w
    if len(widths) >= 2 and widths[-1] < 256:
        pair = widths[-2] + widths[-1]
        a = (pair + 1) // 2
        a = ((a + 31) // 32) * 32          # keep moving-dim nicely aligned
        widths[-2:] = [a, pair - a]
    out = []
    m0 = 0
    for w in widths:
        out.append((m0, w))
        m0 += w
    return out


def _build_program(cap, S):
    nc = bacc.Bacc("TRN2", target_bir_lowering=False, debug=False,
                   num_devices=NCORES)

    xr = nc.declare_dram_parameter("xr", [P, HC, cap], f32r, isOutput=False)
    xs = nc.declare_dram_parameter("xs", [P, HC, S], f32r, isOutput=False)
    wg = nc.declare_dram_parameter("wg", [IC, P, HC, P], f32r, isOutput=False)
    wu = nc.declare_dram_parameter("wu", [IC, P, HC, P], f32r, isOutput=False)
    wd = nc.declare_dram_parameter("wd", [HC, P, IC, P], f32r, isOutput=False)
    sg = nc.declare_dram_parameter("sg", [IC, P, HC, P], f32r, isOutput=False)
    su = nc.declare_dram_parameter("su", [IC, P, HC, P], f32r, isOutput=False)
    sd = nc.declare_dram_parameter("sd", [HC, P, IC, P], f32r, isOutput=False)
    gwt = nc.declare_dram_parameter("gwt", [P, HC, E], f32r, isOutput=False)
    outr = nc.declare_dram_parameter("outr", [H, cap], f32, isOutput=True)
    outs = nc.declare_dram_parameter("outs", [H, S], f32, isOutput=True)
    logt = nc.declare_dram_parameter("logt", [E, S], f32, isOutput=True)

    with tile.TileContext(nc) as tc, ExitStack() as ctx:
        xpool = ctx.enter_context(tc.tile_pool(name="xp", bufs=1))
        w1pool = ctx.enter_context(tc.tile_pool(name="w1", bufs=2))
        hpool = ctx.enter_context(tc.tile_pool(name="hp", bufs=1))
        w2pool = ctx.enter_context(tc.tile_pool(name="w2", bufs=3))
        tpool = ctx.enter_context(tc.tile_pool(name="tp", bufs=3))
        opool = ctx.enter_context(tc.tile_pool(name="op", bufs=3))
        ps1 = ctx.enter_context(tc.tile_pool(name="ps1", bufs=2, space="PSUM"))
        ps2 = ctx.enter_context(tc.tile_pool(name="ps2", bufs=3, space="PSUM"))

        segs = [
            ("r", xr, cap, wg, wu, wd, outr),
            ("s", xs, S, sg, su, sd, outs),
        ]

        for sname, xdram, M, wgd, wud, wdd, odram in segs:
            mts = _m_tiles(M)
            xt = xpool.tile([P, HC, M], f32r, name=f"xt_{sname}", tag="xt")
            for hc in range(HC):
                nc.sync.dma_start(out=xt[:, hc, :], in_=xdram.ap()[:, hc, :])

            ht = hpool.tile([P, IC, M], f32r, name=f"ht_{sname}",
                            tag=f"ht_{sname}")

            # ---- stage 1: g = Wg^T x, u = Wu^T x, ht = silu(g) * u ----
            for ig in range(IC):
                wgt = w1pool.tile([P, HC, P], f32r, name="wgt", tag="wgt")
                wut = w1pool.tile([P, HC, P], f32r, name="wut", tag="wut")
                nc.sync.dma_start(out=wgt, in_=wgd.ap()[ig])
                nc.sync.dma_start(out=wut, in_=wud.ap()[ig])
                for (m0, mw) in mts:
                    pg = ps1.tile([P, 512], f32, name="pg", tag="pg")
                    pu = ps1.tile([P, 512], f32, name="pu", tag="pu")
                    for hc in range(HC):
                        nc.tensor.matmul(pg[:, :mw], lhsT=wgt[:, hc, :],
                                         rhs=xt[:, hc, m0:m0 + mw],
                                         start=(hc == 0), stop=(hc == HC - 1))
                    for hc in range(HC):
                        nc.tensor.matmul(pu[:, :mw], lhsT=wut[:, hc, :],
                                         rhs=xt[:, hc, m0:m0 + mw],
                                         start=(hc == 0), stop=(hc == HC - 1))
                    sil = tpool.tile([P, 512], f32, name="sil", tag="sil")
                    nc.scalar.activation(sil[:, :mw], pg[:, :mw],
                                         mybir.ActivationFunctionType.Silu)
                    nc.vector.tensor_mul(ht[:, ig, m0:m0 + mw], sil[:, :mw],
                                         pu[:, :mw])

            # ---- stage 2: out^T[hj, m] = sum_i Wd[hj, i] * ht[i, m] ----
            for hp in range(HC):
                wdt = w2pool.tile([P, IC, P], f32r, name="wdt", tag="wdt")
                nc.sync.dma_start(out=wdt, in_=wdd.ap()[hp])
                for (m0, mw) in mts:
                    pd = ps2.tile([P, 512], f32, name="pd", tag="pd")
                    for ic in range(IC):
                        nc.tensor.matmul(pd[:, :mw], lhsT=wdt[:, ic, :],
                                         rhs=ht[:, ic, m0:m0 + mw],
                                         start=(ic == 0), stop=(ic == IC - 1))
                    ot = opool.tile([P, 512], f32, name="ot", tag="ot")
                    nc.vector.tensor_copy(ot[:, :mw], pd[:, :mw])
                    nc.sync.dma_start(
                        out=odram.ap()[hp * P:(hp + 1) * P, m0:m0 + mw],
                        in_=ot[:, :mw])

            if sname == "s":
                # ---- router logits for this core's shared token slice ----
                gwtile = xpool.tile([P, HC, E], f32r, name="gwtile",
                                    tag="gwtile")
                nc.sync.dma_start(out=gwtile, in_=gwt.ap())
                for (m0, mw) in mts:
                    pl = ps2.tile([E, 512], f32, name="pl", tag="pd")
                    for hc in range(HC):
                        nc.tensor.matmul(pl[:, :mw], lhsT=gwtile[:, hc, :],
                                         rhs=xt[:, hc, m0:m0 + mw],
                                         start=(hc == 0), stop=(hc == HC - 1))
                    lt = opool.tile([E, 512], f32, name="lt", tag="lt")
                    nc.vector.tensor_copy(lt[:, :mw], pl[:, :mw])
                    nc.sync.dma_start(out=logt.ap()[:, m0:m0 + mw],
                                      in_=lt[:, :mw])

    nc.compile()
    return nc


def _get_program(cap, S):
    key = (cap, S)
    if key not in _prog_cache:
        _prog_cache[key] = _build_program(cap, S)
    return _prog_cache[key]


def _pack_x(seg_x):
    # [M, H] -> [P, HC, M]; [p, hc, m] = x[m, hc*128+p]
    M = seg_x.shape[0]
    return np.ascontiguousarray(seg_x.reshape(M, HC, P).transpose(2, 1, 0))


def _pack_w1(w):
    # [I, H] -> [IC, P(p=h sub), HC, P(i)]; [ig, p, hc, i] = w[ig*128+i, hc*128+p]
    return np.ascontiguousarray(
        w.reshape(IC, P, HC, P).transpose(0, 3, 2, 1))


def _pack_w2(wd_):
    # [H, I] -> [HC, P(pi=i sub), IC, P(hj)]; [hp, pi, ic, hj] = wd[hp*128+hj, ic*128+pi]
    return np.ascontiguousarray(
        wd_.reshape(HC, P, IC, P).transpose(0, 3, 2, 1))


def kernel(hidden_states, gate_w, shared_gate_w, shared_up_w, shared_down_w,
           routed_gate_w, routed_up_w, routed_down_w):
    B, SEQ, Hh = hidden_states.shape
    assert Hh == H
    x = np.ascontiguousarray(hidden_states.reshape(-1, H), dtype=np.float32)
    T = x.shape[0]
    assert T % NCORES == 0
    S = T // NCORES

    # ---- host routing (dispatch) ----
    logits = x @ gate_w.T.astype(np.float32)          # [T, E]
    top_id = logits.argmax(-1)
    top_val = logits.max(-1)
    scale = 1.0 / (1.0 + np.exp(-top_val))
    order = np.argsort(top_id, kind="stable")
    counts = np.bincount(top_id, minlength=E)
    starts = np.zeros(E + 1, np.int64)
    starts[1:] = np.cumsum(counts)
    cap = max(512, int(-(-counts.max() // 128)) * 128)

    sorted_x = x[order]                                # [T, H] unscaled
    sorted_xs = sorted_x * scale[order][:, None]       # [T, H] scaled

    sgP = _pack_w1(np.asarray(shared_gate_w, np.float32))
    suP = _pack_w1(np.asarray(shared_up_w, np.float32))
    sdP = _pack_w2(np.asarray(shared_down_w, np.float32))
    gwP = np.ascontiguousarray(
        np.asarray(gate_w, np.float32).reshape(E, HC, P).transpose(2, 1, 0))

    in_maps = []
    for c in range(NCORES):
        n_c = int(counts[c])
        seg = np.zeros((cap, H), np.float32)
        seg[:n_c] = sorted_xs[starts[c]:starts[c + 1]]
        in_maps.append({
            "xr": _pack_x(seg),
            "xs": _pack_x(sorted_x[c * S:(c + 1) * S]),
            "wg": _pack_w1(np.asarray(routed_gate_w[c], np.float32)),
            "wu": _pack_w1(np.asarray(routed_up_w[c], np.float32)),
            "wd": _pack_w2(np.asarray(routed_down_w[c], np.float32)),
            "sg": sgP,
            "su": suP,
            "sd": sdP,
            "gwt": gwP,
        })

    nc = _get_program(cap, S)
    res = run_bass_kernel_spmd(nc, in_maps, core_ids=list(range(NCORES)))

    # ---- host combine ----
    routed_all = np.concatenate(
        [res.results[e]["outr"].T[:counts[e]] for e in range(E)], axis=0)
    shared_all = np.concatenate(
        [res.results[c]["outs"].T for c in range(NCORES)], axis=0)
    out_flat = np.empty((T, H), np.float32)
    out_flat[order] = routed_all + shared_all

    logt_sorted = np.concatenate(
        [res.results[c]["logt"].T for c in range(NCORES)], axis=0)  # [T, E]
    rl = np.empty((T, E), np.float32)
    rl[order] = logt_sorted

    return out_flat.reshape(B, SEQ, H), rl.reshape(B, SEQ, E)


# revision 7
# speedup vs baseline: 1.0748x; 1.0748x over previous
"""Llama4-style MoE (top-1 routing, E=8) + shared SwiGLU expert on 8 Trainium2 cores.

Strategy (expert-parallel + data-parallel shared, host dispatch/combine):
  - Host computes router logits / top-1 routing / sigmoid scaling (0.03% of FLOPs)
    and sorts tokens by expert — the "dispatch" step of the sharding.
  - Core e gets: the tokens routed to expert e (zero-padded to a uniform CAP),
    expert e's SwiGLU weights, plus a 1/8 slice of all tokens and the replicated
    shared-expert weights.  Each core runs two SwiGLU MLPs (routed segment +
    shared segment) and the router-logits matmul for its shared slice.
  - All matmuls in float32r (fp32 storage, ~bf16 rate on the PE at free>=256).
  - Host packs every device input into the exact SBUF tile layout so each DMA
    is one contiguous >=8KB-per-partition read (DMA packet efficiency).
  - Host "combine": routed and shared partial outputs are summed and scattered
    back to the original token order.

Device layouts (P=128 partitions):
  xt   [P, 16, M]          x^T tokens:      xt[p, hc, m]  = x[m, hc*128+p]
  w1   [16, P, 16, P]      gate/up weights: w1[ig, p, hc, i] = W[ig*128+i, hc*128+p]
  w2   [16, P, 16, P]      down weights:    w2[hp, pi, ic, hj] = Wd[hp*128+hj, ic*128+pi]
  out  [H, M]              output^T (host transposes back)
"""

import numpy as np
from contextlib import ExitStack

import concourse.bacc as bacc
import concourse.tile as tile
from concourse import mybir
from concourse.bass_utils import run_bass_kernel_spmd

P = 128
H = 2048
I = 2048
E = 8
NCORES = 8
HC = H // P    # 16 contraction chunks (stage 1)
IC = I // P    # 16 i chunks

f32 = mybir.dt.float32
f32r = mybir.dt.float32r

_prog_cache: dict = {}


def _m_tiles(M, width=512):
    # float32r matmuls drop to 1/4 rate below a 256-wide moving operand, so
    # keep every tile >= 256 by rebalancing the tail against the previous tile.
    widths = []
    rem = M
    while rem > 0:
        w = min(width, rem)
        widths.append(w)
        rem -= w
    if len(widths) >= 2 and widths[-1] < 256:
        pair = widths[-2] + widths[-1]
        a = (pair + 1) // 2
        a = ((a + 31) // 32) * 32          # keep moving-dim nicely aligned
        widths[-2:] = [a, pair - a]
    out = []
    m0 = 0
    for w in widths:
        out.append((m0, w))
        m0 += w
    return out


def _build_program(cap, S):
    nc = bacc.Bacc("TRN2", target_bir_lowering=False, debug=False,
                   num_devices=NCORES)

    xr = nc.declare_dram_parameter("xr", [P, HC, cap], f32r, isOutput=False)
    xs = nc.declare_dram_parameter("xs", [P, HC, S], f32r, isOutput=False)
    wg = nc.declare_dram_parameter("wg", [IC, P, HC, P], f32r, isOutput=False)
    wu = nc.declare_dram_parameter("wu", [IC, P, HC, P], f32r, isOutput=False)
    wd = nc.declare_dram_parameter("wd", [HC, P, IC, P], f32r, isOutput=False)
    sg = nc.declare_dram_parameter("sg", [IC, P, HC, P], f32r, isOutput=False)
    su = nc.declare_dram_parameter("su", [IC, P, HC, P], f32r, isOutput=False)
    sd = nc.declare_dram_parameter("sd", [HC, P, IC, P], f32r, isOutput=False)
    gwt = nc.declare_dram_parameter("gwt", [P, HC, E], f32r, isOutput=False)
    outr = nc.declare_dram_parameter("outr", [H, cap], f32, isOutput=True)
    outs = nc.declare_dram_parameter("outs", [H, S], f32, isOutput=True)
    logt = nc.declare_dram_parameter("logt", [E, S], f32, isOutput=True)

    with tile.TileContext(nc) as tc, ExitStack() as ctx:
        xpool = ctx.enter_context(tc.tile_pool(name="xp", bufs=1))
        w1pool = ctx.enter_context(tc.tile_pool(name="w1", bufs=2))
        hpool = ctx.enter_context(tc.tile_pool(name="hp", bufs=1))
        w2pool = ctx.enter_context(tc.tile_pool(name="w2", bufs=3))
        tpool = ctx.enter_context(tc.tile_pool(name="tp", bufs=3))
        opool = ctx.enter_context(tc.tile_pool(name="op", bufs=3))
        # PSUM budget (8 banks): stage1 holds one accumulation chain per
        # (g/u, m-tile) across the whole contraction loop so each LDWEIGHTS
        # (expensive for 4-byte dtypes) serves len(mts) matmuls.
        psum = ctx.enter_context(tc.tile_pool(name="psum", bufs=1,
                                              space="PSUM"))

        segs = [
            ("r", xr, cap, wg, wu, wd, outr),
            ("s", xs, S, sg, su, sd, outs),
        ]

        for sname, xdram, M, wgd, wud, wdd, odram in segs:
            mts = _m_tiles(M)
            xt = xpool.tile([P, HC, M], f32r, name=f"xt_{sname}", tag="xt")
            for hc in range(HC):
                nc.sync.dma_start(out=xt[:, hc, :], in_=xdram.ap()[:, hc, :])

            ht = hpool.tile([P, IC, M], f32r, name=f"ht_{sname}",
                            tag=f"ht_{sname}")

            # ---- stage 1: g = Wg^T x, u = Wu^T x, ht = silu(g) * u ----
            for ig in range(IC):
                wgt = w1pool.tile([P, HC, P], f32r, name="wgt", tag="wgt")
                wut = w1pool.tile([P, HC, P], f32r, name="wut", tag="wut")
                nc.sync.dma_start(out=wgt, in_=wgd.ap()[ig])
                nc.sync.dma_start(out=wut, in_=wud.ap()[ig])
                pgs = [psum.tile([P, 512], f32, name=f"pg{k}", tag=f"pg{k}")
                       for k in range(len(mts))]
                pus = [psum.tile([P, 512], f32, name=f"pu{k}", tag=f"pu{k}")
                       for k in range(len(mts))]
                for hc in range(HC):
                    for k, (m0, mw) in enumerate(mts):
                        nc.tensor.matmul(pgs[k][:, :mw], lhsT=wgt[:, hc, :],
                                         rhs=xt[:, hc, m0:m0 + mw],
                                         start=(hc == 0), stop=(hc == HC - 1))
                    for k, (m0, mw) in enumerate(mts):
                        nc.tensor.matmul(pus[k][:, :mw], lhsT=wut[:, hc, :],
                                         rhs=xt[:, hc, m0:m0 + mw],
                                         start=(hc == 0), stop=(hc == HC - 1))
                for k, (m0, mw) in enumerate(mts):
                    sil = tpool.tile([P, 512], f32, name="sil", tag="sil")
                    nc.scalar.activation(sil[:, :mw], pgs[k][:, :mw],
                                         mybir.ActivationFunctionType.Silu)
                    nc.vector.tensor_mul(ht[:, ig, m0:m0 + mw], sil[:, :mw],
                                         pus[k][:, :mw])

            # ---- stage 2: out^T[hj, m] = sum_i Wd[hj, i] * ht[i, m] ----
            for hp in range(HC):
                wdt = w2pool.tile([P, IC, P], f32r, name="wdt", tag="wdt")
                nc.sync.dma_start(out=wdt, in_=wdd.ap()[hp])
                pds = [psum.tile([P, 512], f32, name=f"pd{k}", tag=f"pd{k}")
                       for k in range(len(mts))]
                for ic in range(IC):
                    for k, (m0, mw) in enumerate(mts):
                        nc.tensor.matmul(pds[k][:, :mw], lhsT=wdt[:, ic, :],
                                         rhs=ht[:, ic, m0:m0 + mw],
                                         start=(ic == 0), stop=(ic == IC - 1))
                for k, (m0, mw) in enumerate(mts):
                    ot = opool.tile([P, 512], f32, name="ot", tag="ot")
                    nc.vector.tensor_copy(ot[:, :mw], pds[k][:, :mw])
                    nc.sync.dma_start(
                        out=odram.ap()[hp * P:(hp + 1) * P, m0:m0 + mw],
                        in_=ot[:, :mw])

            if sname == "s":
                # ---- router logits for this core's shared token slice ----
                gwtile = xpool.tile([P, HC, E], f32r, name="gwtile",
                                    tag="gwtile")
                nc.sync.dma_start(out=gwtile, in_=gwt.ap())
                for (m0, mw) in mts:
                    pl = psum.tile([E, 512], f32, name="pl", tag="pd0")
                    for hc in range(HC):
                        nc.tensor.matmul(pl[:, :mw], lhsT=gwtile[:, hc, :],
                                         rhs=xt[:, hc, m0:m0 + mw],
                                         start=(hc == 0), stop=(hc == HC - 1))
                    lt = opool.tile([E, 512], f32, name="lt", tag="lt")
                    nc.vector.tensor_copy(lt[:, :mw], pl[:, :mw])
                    nc.sync.dma_start(out=logt.ap()[:, m0:m0 + mw],
                                      in_=lt[:, :mw])

    nc.compile()
    return nc


def _get_program(cap, S):
    key = (cap, S)
    if key not in _prog_cache:
        _prog_cache[key] = _build_program(cap, S)
    return _prog_cache[key]


def _pack_x(seg_x):
    # [M, H] -> [P, HC, M]; [p, hc, m] = x[m, hc*128+p]
    M = seg_x.shape[0]
    return np.ascontiguousarray(seg_x.reshape(M, HC, P).transpose(2, 1, 0))


def _pack_w1(w):
    # [I, H] -> [IC, P(p=h sub), HC, P(i)]; [ig, p, hc, i] = w[ig*128+i, hc*128+p]
    return np.ascontiguousarray(
        w.reshape(IC, P, HC, P).transpose(0, 3, 2, 1))


def _pack_w2(wd_):
    # [H, I] -> [HC, P(pi=i sub), IC, P(hj)]; [hp, pi, ic, hj] = wd[hp*128+hj, ic*128+pi]
    return np.ascontiguousarray(
        wd_.reshape(HC, P, IC, P).transpose(0, 3, 2, 1))


def kernel(hidden_states, gate_w, shared_gate_w, shared_up_w, shared_down_w,
           routed_gate_w, routed_up_w, routed_down_w):
    B, SEQ, Hh = hidden_states.shape
    assert Hh == H
    x = np.ascontiguousarray(hidden_states.reshape(-1, H), dtype=np.float32)
    T = x.shape[0]
    assert T % NCORES == 0
    S = T // NCORES

    # ---- host routing (dispatch) ----
    logits = x @ gate_w.T.astype(np.float32)          # [T, E]
    top_id = logits.argmax(-1)
    top_val = logits.max(-1)
    scale = 1.0 / (1.0 + np.exp(-top_val))
    order = np.argsort(top_id, kind="stable")
    counts = np.bincount(top_id, minlength=E)
    starts = np.zeros(E + 1, np.int64)
    starts[1:] = np.cumsum(counts)
    cap = max(512, int(-(-counts.max() // 128)) * 128)

    sorted_x = x[order]                                # [T, H] unscaled
    sorted_xs = sorted_x * scale[order][:, None]       # [T, H] scaled

    sgP = _pack_w1(np.asarray(shared_gate_w, np.float32))
    suP = _pack_w1(np.asarray(shared_up_w, np.float32))
    sdP = _pack_w2(np.asarray(shared_down_w, np.float32))
    gwP = np.ascontiguousarray(
        np.asarray(gate_w, np.float32).reshape(E, HC, P).transpose(2, 1, 0))

    in_maps = []
    for c in range(NCORES):
        n_c = int(counts[c])
        seg = np.zeros((cap, H), np.float32)
        seg[:n_c] = sorted_xs[starts[c]:starts[c + 1]]
        in_maps.append({
            "xr": _pack_x(seg),
            "xs": _pack_x(sorted_x[c * S:(c + 1) * S]),
            "wg": _pack_w1(np.asarray(routed_gate_w[c], np.float32)),
            "wu": _pack_w1(np.asarray(routed_up_w[c], np.float32)),
            "wd": _pack_w2(np.asarray(routed_down_w[c], np.float32)),
            "sg": sgP,
            "su": suP,
            "sd": sdP,
            "gwt": gwP,
        })

    nc = _get_program(cap, S)
    res = run_bass_kernel_spmd(nc, in_maps, core_ids=list(range(NCORES)))

    # ---- host combine ----
    routed_all = np.concatenate(
        [res.results[e]["outr"].T[:counts[e]] for e in range(E)], axis=0)
    shared_all = np.concatenate(
        [res.results[c]["outs"].T for c in range(NCORES)], axis=0)
    out_flat = np.empty((T, H), np.float32)
    out_flat[order] = routed_all + shared_all

    logt_sorted = np.concatenate(
        [res.results[c]["logt"].T for c in range(NCORES)], axis=0)  # [T, E]
    rl = np.empty((T, E), np.float32)
    rl[order] = logt_sorted

    return out_flat.reshape(B, SEQ, H), rl.reshape(B, SEQ, E)


# revision 8
# speedup vs baseline: 1.1396x; 1.0603x over previous
"""Llama4-style MoE (top-1 routing, E=8) + shared SwiGLU expert on 8 Trainium2 cores.

Strategy (expert-parallel + data-parallel shared, host dispatch/combine):
  - Host computes router logits / top-1 routing / sigmoid scaling (0.03% of FLOPs)
    and sorts tokens by expert — the "dispatch" step of the sharding.
  - Core e gets: the tokens routed to expert e (zero-padded to a uniform CAP),
    expert e's SwiGLU weights, plus a 1/8 slice of all tokens and the replicated
    shared-expert weights.  Each core runs two SwiGLU MLPs (routed segment +
    shared segment) and the router-logits matmul for its shared slice.
  - Host packs every device input into the exact SBUF tile layout so each DMA
    is one contiguous >=2KB-per-partition read (DMA packet efficiency).
  - Host "combine": routed and shared partial outputs are summed and scattered
    back to the original token order.

Device layouts (P=128 partitions):
  xt   [P, 16, M]          x^T tokens:      xt[p, hc, m]  = x[m, hc*128+p]
  w1   [16, P, 16, P]      gate/up weights: w1[ig, p, hc, i] = W[ig*128+i, hc*128+p]
  w2   [16, P, 16, P]      down weights:    w2[hp, pi, ic, hj] = Wd[hp*128+hj, ic*128+pi]
  out  [H, M]              output^T (host transposes back)

MM_DTYPE selects matmul operand precision:
  "f32r": fp32 storage, ~1cyc/row PE rate but 4-byte LDWEIGHTS (~190ns) drags
          narrow-moving matmuls; rel err ~2.5e-4.
  "bf16": half the weight DMA, single-pass LDWEIGHTS (+FWL); rel err ~5e-3.
"""

import os
import numpy as np
import ml_dtypes
from contextlib import ExitStack

import concourse.bacc as bacc
import concourse.tile as tile
from concourse import mybir
from concourse.bass_utils import run_bass_kernel_spmd

P = 128
H = 2048
I = 2048
E = 8
NCORES = 8
HC = H // P    # 16 contraction chunks (stage 1)
IC = I // P    # 16 i chunks

MM_DTYPE = os.environ.get("MOE_MM_DTYPE", "f32r")

f32 = mybir.dt.float32

_prog_cache: dict = {}


def _m_tiles(M, width=512):
    # float32r matmuls pay their LDWEIGHTS serially below ~450-wide moving
    # operands; balanced tiles also keep every tile >= 256.
    widths = []
    rem = M
    while rem > 0:
        w = min(width, rem)
        widths.append(w)
        rem -= w
    if len(widths) >= 2 and widths[-1] < 256:
        pair = widths[-2] + widths[-1]
        a = (pair + 1) // 2
        a = ((a + 31) // 32) * 32          # keep moving-dim nicely aligned
        widths[-2:] = [a, pair - a]
    out = []
    m0 = 0
    for w in widths:
        out.append((m0, w))
        m0 += w
    return out


def _build_program(cap, S, mm_dtype):
    mmdt = mybir.dt.float32r if mm_dtype == "f32r" else mybir.dt.bfloat16
    nc = bacc.Bacc("TRN2", target_bir_lowering=False, debug=False,
                   num_devices=NCORES)

    xr = nc.declare_dram_parameter("xr", [P, HC, cap], mmdt, isOutput=False)
    xs = nc.declare_dram_parameter("xs", [P, HC, S], mmdt, isOutput=False)
    wg = nc.declare_dram_parameter("wg", [IC, P, HC, P], mmdt, isOutput=False)
    wu = nc.declare_dram_parameter("wu", [IC, P, HC, P], mmdt, isOutput=False)
    wd = nc.declare_dram_parameter("wd", [HC, P, IC, P], mmdt, isOutput=False)
    sg = nc.declare_dram_parameter("sg", [IC, P, HC, P], mmdt, isOutput=False)
    su = nc.declare_dram_parameter("su", [IC, P, HC, P], mmdt, isOutput=False)
    sd = nc.declare_dram_parameter("sd", [HC, P, IC, P], mmdt, isOutput=False)
    gwt = nc.declare_dram_parameter("gwt", [P, HC, E], mmdt, isOutput=False)
    outr = nc.declare_dram_parameter("outr", [H, cap], f32, isOutput=True)
    outs = nc.declare_dram_parameter("outs", [H, S], f32, isOutput=True)
    logt = nc.declare_dram_parameter("logt", [E, S], f32, isOutput=True)

    with tile.TileContext(nc) as tc, ExitStack() as ctx:
        xpool = ctx.enter_context(tc.tile_pool(name="xp", bufs=1))
        w1pool = ctx.enter_context(tc.tile_pool(name="w1", bufs=2))
        hpool = ctx.enter_context(tc.tile_pool(name="hp", bufs=1))
        w2pool = ctx.enter_context(tc.tile_pool(name="w2", bufs=3))
        tpool = ctx.enter_context(tc.tile_pool(name="tp", bufs=3))
        opool = ctx.enter_context(tc.tile_pool(name="op", bufs=3))
        # PSUM (8 banks): stage1 holds one accumulation chain per (g/u, m-tile)
        # across the whole contraction loop so each LDWEIGHTS serves len(mts)
        # matmuls; chains ping-pong across loop iterations to hide eviction.
        psum = ctx.enter_context(tc.tile_pool(name="psum", bufs=1,
                                              space="PSUM"))

        def s1_tags(n_mts, ig):
            if n_mts == 1:
                return [f"pg{ig % 2}"], [f"pu{ig % 2}"]
            return [f"pg{k}" for k in range(n_mts)], \
                   [f"pu{k}" for k in range(n_mts)]

        def s2_tags(n_mts, hp):
            if n_mts == 1:
                return [f"pd{hp % 4}"]
            base = 2 * (hp % 2)
            return [f"pd{base + k}" for k in range(n_mts)]

        def stage1(xt, ht, wgd, wud, mts, preload):
            for ig in range(IC):
                if ig == 0 and preload is not None:
                    wgt, wut = preload
                else:
                    wgt = w1pool.tile([P, HC, P], mmdt, name="wgt", tag="wgt")
                    wut = w1pool.tile([P, HC, P], mmdt, name="wut", tag="wut")
                    nc.sync.dma_start(out=wgt, in_=wgd.ap()[ig])
                    nc.sync.dma_start(out=wut, in_=wud.ap()[ig])
                gtags, utags = s1_tags(len(mts), ig)
                pgs = [psum.tile([P, 512], f32, name=t, tag=t) for t in gtags]
                pus = [psum.tile([P, 512], f32, name=t, tag=t) for t in utags]
                for hc in range(HC):
                    for k, (m0, mw) in enumerate(mts):
                        nc.tensor.matmul(pgs[k][:, :mw], lhsT=wgt[:, hc, :],
                                         rhs=xt[:, hc, m0:m0 + mw],
                                         start=(hc == 0), stop=(hc == HC - 1))
                    for k, (m0, mw) in enumerate(mts):
                        nc.tensor.matmul(pus[k][:, :mw], lhsT=wut[:, hc, :],
                                         rhs=xt[:, hc, m0:m0 + mw],
                                         start=(hc == 0), stop=(hc == HC - 1))
                for k, (m0, mw) in enumerate(mts):
                    sil = tpool.tile([P, 512], f32, name="sil", tag="sil")
                    nc.scalar.activation(sil[:, :mw], pgs[k][:, :mw],
                                         mybir.ActivationFunctionType.Silu)
                    nc.vector.tensor_mul(ht[:, ig, m0:m0 + mw], sil[:, :mw],
                                         pus[k][:, :mw])

        def stage2(ht, wdd, odram, mts):
            for hp in range(HC):
                wdt = w2pool.tile([P, IC, P], mmdt, name="wdt", tag="wdt")
                nc.sync.dma_start(out=wdt, in_=wdd.ap()[hp])
                tags = s2_tags(len(mts), hp)
                pds = [psum.tile([P, 512], f32, name=t, tag=t) for t in tags]
                for ic in range(IC):
                    for k, (m0, mw) in enumerate(mts):
                        nc.tensor.matmul(pds[k][:, :mw], lhsT=wdt[:, ic, :],
                                         rhs=ht[:, ic, m0:m0 + mw],
                                         start=(ic == 0), stop=(ic == IC - 1))
                for k, (m0, mw) in enumerate(mts):
                    ot = opool.tile([P, 512], f32, name="ot", tag="ot")
                    nc.vector.tensor_copy(ot[:, :mw], pds[k][:, :mw])
                    nc.sync.dma_start(
                        out=odram.ap()[hp * P:(hp + 1) * P, m0:m0 + mw],
                        in_=ot[:, :mw])

        segs = [
            ("r", xr, cap, wg, wu, wd, outr),
            ("s", xs, S, sg, su, sd, outs),
        ]

        for sname, xdram, M, wgd, wud, wdd, odram in segs:
            mts = _m_tiles(M)
            # First gate/up weight block before the (larger) activation load
            # so the PE's first accumulation chain can start ~6us earlier.
            wgt0 = w1pool.tile([P, HC, P], mmdt, name="wgt", tag="wgt")
            wut0 = w1pool.tile([P, HC, P], mmdt, name="wut", tag="wut")
            nc.sync.dma_start(out=wgt0, in_=wgd.ap()[0])
            nc.sync.dma_start(out=wut0, in_=wud.ap()[0])

            xt = xpool.tile([P, HC, M], mmdt, name=f"xt_{sname}", tag="xt")
            for hc in range(HC):
                nc.sync.dma_start(out=xt[:, hc, :], in_=xdram.ap()[:, hc, :])

            ht = hpool.tile([P, IC, M], mmdt, name=f"ht_{sname}",
                            tag=f"ht_{sname}")

            stage1(xt, ht, wgd, wud, mts, (wgt0, wut0))

            if sname == "s":
                # Router logits for this core's shared token slice; emitted
                # before stage2 so its latency hides under the down-proj.
                gwtile = xpool.tile([P, HC, E], mmdt, name="gwtile",
                                    tag="gwtile")
                nc.sync.dma_start(out=gwtile, in_=gwt.ap())
                for mi, (m0, mw) in enumerate(mts):
                    pl = psum.tile([E, 512], f32, name="pl", tag=f"pd{mi % 4}")
                    for hc in range(HC):
                        nc.tensor.matmul(pl[:, :mw], lhsT=gwtile[:, hc, :],
                                         rhs=xt[:, hc, m0:m0 + mw],
                                         start=(hc == 0), stop=(hc == HC - 1))
                    lt = opool.tile([E, 512], f32, name="lt", tag="lt")
                    nc.vector.tensor_copy(lt[:, :mw], pl[:, :mw])
                    nc.sync.dma_start(out=logt.ap()[:, m0:m0 + mw],
                                      in_=lt[:, :mw])

            stage2(ht, wdd, odram, mts)

    nc.compile()
    return nc


def _get_program(cap, S, mm_dtype):
    key = (cap, S, mm_dtype)
    if key not in _prog_cache:
        _prog_cache[key] = _build_program(cap, S, mm_dtype)
    return _prog_cache[key]


def _mmnp(a):
    if MM_DTYPE == "f32r":
        return np.ascontiguousarray(a, dtype=np.float32)
    return np.ascontiguousarray(a.astype(ml_dtypes.bfloat16))


def _pack_x(seg_x):
    # [M, H] -> [P, HC, M]; [p, hc, m] = x[m, hc*128+p]
    M = seg_x.shape[0]
    return _mmnp(seg_x.reshape(M, HC, P).transpose(2, 1, 0))


def _pack_w1(w):
    # [I, H] -> [IC, P(p=h sub), HC, P(i)]; [ig, p, hc, i] = w[ig*128+i, hc*128+p]
    return _mmnp(w.reshape(IC, P, HC, P).transpose(0, 3, 2, 1))


def _pack_w2(wd_):
    # [H, I] -> [HC, P(pi=i sub), IC, P(hj)]; [hp, pi, ic, hj] = wd[hp*128+hj, ic*128+pi]
    return _mmnp(wd_.reshape(HC, P, IC, P).transpose(0, 3, 2, 1))


def kernel(hidden_states, gate_w, shared_gate_w, shared_up_w, shared_down_w,
           routed_gate_w, routed_up_w, routed_down_w):
    B, SEQ, Hh = hidden_states.shape
    assert Hh == H
    x = np.ascontiguousarray(hidden_states.reshape(-1, H), dtype=np.float32)
    T = x.shape[0]
    assert T % NCORES == 0
    S = T // NCORES

    # ---- host routing (dispatch) ----
    logits = x @ gate_w.T.astype(np.float32)          # [T, E]
    top_id = logits.argmax(-1)
    top_val = logits.max(-1)
    scale = 1.0 / (1.0 + np.exp(-top_val))
    order = np.argsort(top_id, kind="stable")
    counts = np.bincount(top_id, minlength=E)
    starts = np.zeros(E + 1, np.int64)
    starts[1:] = np.cumsum(counts)
    cap = max(512, int(-(-counts.max() // 128)) * 128)

    sorted_x = x[order]                                # [T, H] unscaled
    sorted_xs = sorted_x * scale[order][:, None]       # [T, H] scaled

    sgP = _pack_w1(np.asarray(shared_gate_w, np.float32))
    suP = _pack_w1(np.asarray(shared_up_w, np.float32))
    sdP = _pack_w2(np.asarray(shared_down_w, np.float32))
    gwP = _mmnp(np.asarray(gate_w, np.float32).reshape(E, HC, P)
                .transpose(2, 1, 0))

    in_maps = []
    for c in range(NCORES):
        n_c = int(counts[c])
        seg = np.zeros((cap, H), np.float32)
        seg[:n_c] = sorted_xs[starts[c]:starts[c + 1]]
        in_maps.append({
            "xr": _pack_x(seg),
            "xs": _pack_x(sorted_x[c * S:(c + 1) * S]),
            "wg": _pack_w1(np.asarray(routed_gate_w[c], np.float32)),
            "wu": _pack_w1(np.asarray(routed_up_w[c], np.float32)),
            "wd": _pack_w2(np.asarray(routed_down_w[c], np.float32)),
            "sg": sgP,
            "su": suP,
            "sd": sdP,
            "gwt": gwP,
        })

    nc = _get_program(cap, S, MM_DTYPE)
    res = run_bass_kernel_spmd(nc, in_maps, core_ids=list(range(NCORES)))

    # ---- host combine ----
    routed_all = np.concatenate(
        [res.results[e]["outr"].T[:counts[e]] for e in range(E)], axis=0)
    shared_all = np.concatenate(
        [res.results[c]["outs"].T for c in range(NCORES)], axis=0)
    out_flat = np.empty((T, H), np.float32)
    out_flat[order] = routed_all + shared_all

    logt_sorted = np.concatenate(
        [res.results[c]["logt"].T for c in range(NCORES)], axis=0)  # [T, E]
    rl = np.empty((T, E), np.float32)
    rl[order] = logt_sorted

    return out_flat.reshape(B, SEQ, H), rl.reshape(B, SEQ, E)
